# revision 29
# baseline (speedup 1.0000x reference)
"""Causal self-attention (B=2, T=2048, C=1024, H=16, D=64) on 8 TRN2 cores.

Sharding: batch across 2 groups of 4 cores; 4 heads per core within a group
(Megatron column-parallel QKV). After attention, AllGather the per-head
outputs within each group, then column-parallel c_proj (each core computes
256 output columns for all T), so the device program is rank-independent.

Per-core dataflow (all matmul operands float32r = full-rate fp32; bf16 was
tried and loses ~30us: this toolchain splits non-fp32 matmuls into explicit
Ldweights+Matmult pairs whose weight loads do not overlap the stream):
  xt  [128, 8, 2048]  x[b]^T chunked by contraction (C) blocks
  Q^T/K^T computed as [512 rows, T] (lhsT = w_qk slices, rhs = xt).
  V computed natural [T, 4 heads x 128] where columns 0:64 of each head's
  block are ONES (memset) — the att@V matmul then emits the softmax
  denominator replicated across psum rows 0:64 and y^T in rows 64:128, so
  normalization is a pure-DVE reciprocal + fused multiply with no PE
  broadcast matmuls and no denominator DMA round-trip (the ones come
  first so the denominator sits at psum base partition 0, where the
  base-partition-matched DVE reciprocal can read it).
  S^T block matmuls (K=64) row-paired across head pairs (partitions 0-63 /
  64-127), exp on ACT (scale=1/8 fused), triangular mask on diagonal
  128-blocks. fp32r matmuls with moving dim < 256 run at 1/4 rate, so the
  last diagonal tile's score/att@V streams are widened from 128 to 256
  (junk score columns are zeroed in es before att@V accumulates them).

QKV (stage A) and attention (stage B) are emitted interleaved per t-chunk
so the Tile scheduler can fill PE gaps during ACT exp with next-chunk QKV
matmuls. Each t-chunk's DMA+AllGather is deferred by one chunk and issued
mid-compute so the first three AllGathers hide under attention. The
score/accumulator PSUM pools close right after attention so proj quarters
0-2 run before the final AllGather, leaving only quarter 3's proj on the
exposed tail.

Output per core: out_c [256, 2048] = out^T columns slice; host reassembles.
"""

import sys

sys.path.insert(0, "/opt/trn_rl_repo")

from contextlib import ExitStack

import numpy as np

B, T, C, H, D = 2, 2048, 1024, 16, 64
NCORES = 8
HL = 4  # heads per core
NKC = 8  # contraction chunks (C / 128)
NCH = 4  # t chunks (T / 512)
NST = 16  # s tiles (T / 128)

_prog_cache = {}

# When True, AllGather is replaced by local DRAM copies (same deps/bytes
# shape) so the single-core TimelineSim can analyze the program.
SIM_NO_COLLECTIVE = False

# Normalize implementation: 'approx' (custom-DVE reciprocal + mult),
# 'recip' (native DVE reciprocal + mult), 'divide' (den copy + ALU divide),
# 'copy' (no normalize — WRONG RESULTS, timing probe only).
NORM_MODE = "approx"

# Widen diag-3 fp32r streams from 128 to 256 (with es zero-memset).
WIDEN = 1

# Timing probes: att@V stationary columns (128 = ones+V, 65 = probe);
# skip the v_sb ones memset (wrong results, timing probe).
AVM = 128
MEMSET = 1

# bf16 attention: qk/es/v/y/proj-weight tensors in bf16 (QKV and V
# projections stay fp32r). Measured faster than fp32r on hw despite
# the explicit Ldweights the toolchain emits for non-fp32 matmuls.
ATT_BF16 = 1

# bf16 QKV/V projections too: xt, wqk, wv in bf16 (halves their SBUF
# streams and DMA). Costs ~0.5% relative error on q/k/v.
QKV_BF16 = 1



def build_program(reps=1, qk_bias=False, out_bias=False):
    key = (reps, qk_bias, out_bias, SIM_NO_COLLECTIVE, NORM_MODE, WIDEN,
           AVM, MEMSET, ATT_BF16, QKV_BF16)
    if key in _prog_cache:
        return _prog_cache[key]

    from concourse import bacc, mybir
    import concourse.tile as tile

    F32 = mybir.dt.float32
    F32R = mybir.dt.float32r
    BF16 = mybir.dt.bfloat16

    nc = bacc.Bacc(num_devices=NCORES)

    ADT = BF16 if ATT_BF16 else F32R
    QDT = BF16 if QKV_BF16 else F32R
    xt = nc.declare_dram_parameter("xt", [128, NKC, T], QDT, isOutput=False)
    wqk = nc.declare_dram_parameter("wqk", [128, NKC, 512], QDT, isOutput=False)
    wv = nc.declare_dram_parameter("wv", [128, NKC, 256], QDT, isOutput=False)
    wp = nc.declare_dram_parameter("wp", [128, NKC, 256], ADT, isOutput=False)
    tri = nc.declare_dram_parameter("tri", [128, 128], ADT, isOutput=False)
    if qk_bias:
        bqk = nc.declare_dram_parameter("bqk", [128, 4], F32, isOutput=False)
    if out_bias:
        bout = nc.declare_dram_parameter("bout", [128, 2], F32, isOutput=False)
    out_c = nc.declare_dram_parameter("out_c", [256, T], F32, isOutput=True)

    with tile.TileContext(nc) as tc:
        with ExitStack() as outer:
            const = outer.enter_context(tc.tile_pool(name="const", bufs=1))
            wqk_sb = const.tile([128, NKC, 512], QDT)
            wv_sb = const.tile([128, NKC, 256], QDT)
            wp_sb = const.tile([128, NKC, 256], ADT)
            tri_sb = const.tile([128, 128], ADT)
            nc.scalar.dma_start(wqk_sb[:], wqk[:])
            nc.scalar.dma_start(wv_sb[:], wv[:])
            nc.scalar.dma_start(wp_sb[:], wp[:])
            nc.scalar.dma_start(tri_sb[:], tri[:])
            bqk_sb = bout_sb = None
            if qk_bias:
                bqk_sb = const.tile([128, 4], F32)
                nc.scalar.dma_start(bqk_sb[:], bqk[:])
            if out_bias:
                bout_sb = const.tile([128, 2], F32)
                nc.scalar.dma_start(bout_sb[:], bout[:])

            for rep in range(reps):
                _emit_body(
                    nc, tc, mybir, rep,
                    xt=xt, out_c=out_c,
                    wqk_sb=wqk_sb, wv_sb=wv_sb, wp_sb=wp_sb, tri_sb=tri_sb,
                    bqk_sb=bqk_sb, bout_sb=bout_sb,
                )

    nc.finalize()
    _prog_cache[key] = nc
    return nc


def _emit_body(nc, tc, mybir, rep, *, xt, out_c, wqk_sb, wv_sb, wp_sb,
               tri_sb, bqk_sb, bout_sb):
    F32 = mybir.dt.float32
    F32R = mybir.dt.float32r
    BF16 = mybir.dt.bfloat16
    ADT = BF16 if ATT_BF16 else F32R
    QDT = BF16 if QKV_BF16 else F32R
    AF = mybir.ActivationFunctionType
    MUL = mybir.AluOpType.mult
    R = f"r{rep}"

    with ExitStack() as persist:
        stP = persist.enter_context(tc.tile_pool(name=f"stP{R}", bufs=1))
        dpool = persist.enter_context(
            tc.tile_pool(name=f"dram{R}", bufs=1, space="DRAM"))
        # Q^T/K^T: m-tiles 0,1 = Q pairs; 2,3 = K pairs. [128, m, t]
        qk_sb = stP.tile([128, 4, T], ADT, name=f"qk_sb{R}")
        # V natural, per-rep (a shared tile would serialize reps); per head
        # 128 cols: 0:64 = ones (softmax denominator via the att@V matmul,
        # landing at psum base partition 0 where the base-matched DVE
        # reciprocal can read it), 64:128 = V values.
        v_sb = stP.tile([128, NST, 4, 128], ADT, name=f"v_sb{R}")
        y_in_q = [
            dpool.tile([256, 512], ADT, name=f"y_in{R}_{q}")
            for q in range(NCH)
        ]
        y_full_q = [
            dpool.tile([1024, 512], ADT, name=f"y_full{R}_{q}")
            for q in range(NCH)
        ]

        with (
            tc.tile_pool(name=f"stAB{R}", bufs=1) as stAB,
            tc.tile_pool(name=f"psA{R}", bufs=1, space="PSUM") as psA,
        ):
            sy_ctx = ExitStack()
            psS = sy_ctx.enter_context(
                tc.tile_pool(name=f"psS{R}", bufs=1, space="PSUM"))
            psY = sy_ctx.enter_context(
                tc.tile_pool(name=f"psY{R}", bufs=1, space="PSUM"))
            if MEMSET:
                ones = v_sb[:, :, :, 0:64]
                nc.vector.memset(
                    ones if ATT_BF16 else ones.bitcast(F32), 1.0)
            xt_t = []
            for n in range(NCH):
                xtile = stAB.tile([128, NKC, 512], QDT, tag="xt", bufs=3,
                                  name=f"xt_t{R}_{n}")
                nc.sync.dma_start(
                    xtile[:], xt[:, :, n * 512:(n + 1) * 512])
                xt_t.append(xtile)

            def emit_ag(n, ynorm):
                nc.scalar.dma_start(
                    y_in_q[n][:].rearrange("(h p) u -> p h u", p=64),
                    ynorm[:],
                )
                if SIM_NO_COLLECTIVE:
                    for r in range(4):
                        nc.gpsimd.dma_start(
                            y_full_q[n][r * 256:(r + 1) * 256, :],
                            y_in_q[n][:],
                        )
                else:
                    nc.gpsimd.collective_compute(
                        "AllGather",
                        mybir.AluOpType.bypass,
                        replica_groups=[[0, 1, 2, 3], [4, 5, 6, 7]],
                        ins=[y_in_q[n][:]],
                        outs=[y_full_q[n][:]],
                    )

            pending = None
            for n in range(NCH):
                    # normalized y^T, heads on free dim: [64 d, 4 h, 512 t];
                    # written this chunk, DMA'd next chunk (2 rotating bufs)
                    ynorm = stAB.tile([64, 4, 512], ADT, tag="yn", bufs=2,
                                      name=f"ynorm{R}_{n}")
                    for m in range(4):
                        ps = psA.tile([128, 512], F32, tag="pA", bufs=2,
                                      name=f"qkvps{R}_{n}_{m}")
                        for kc in range(NKC):
                            nc.tensor.matmul(
                                ps[:],
                                wqk_sb[:, kc, m * 128:(m + 1) * 128],
                                xt_t[n][:, kc, :],
                                start=(kc == 0), stop=(kc == NKC - 1),
                            )
                        if bqk_sb is not None:
                            nc.scalar.activation(
                                qk_sb[:, m, n * 512:(n + 1) * 512], ps[:],
                                AF.Copy, bias=bqk_sb[:, m:m + 1],
                            )
                        else:
                            nc.vector.tensor_copy(
                                qk_sb[:, m, n * 512:(n + 1) * 512], ps[:]
                            )
                    for tt in range(4 * n, 4 * n + 4):
                        psv = psA.tile([128, 512], F32, tag="pA", bufs=2,
                                       name=f"vps{R}_{tt}")
                        for kc in range(NKC):
                            nc.tensor.matmul(
                                psv[:, 0:256],
                                xt_t[n][:, kc,
                                        (tt - 4 * n) * 128:
                                        (tt - 4 * n + 1) * 128],
                                wv_sb[:, kc, :],
                                start=(kc == 0), stop=(kc == NKC - 1),
                            )
                        nc.vector.tensor_copy(
                            v_sb[:, tt, :, 64:128],
                            psv[:, 0:256].rearrange("p (h x) -> p h x", h=4),
                        )

                    if pending is not None:
                        emit_ag(*pending)
                        pending = None

                    n_st = 4 * n + 4
                    for p in range(2):
                        ype = psY.tile([128, 512], F32, tag="ye", bufs=1,
                                       name=f"ype{R}_{n}_{p}")
                        ypo = psY.tile([128, 512], F32, tag="yo", bufs=1,
                                       name=f"ypo{R}_{n}_{p}")
                        for st in range(n_st):
                            diag = st - 4 * n
                            toff = 128 * diag if diag >= 0 else 0
                            # fp32r matmuls with moving dim < 256 run at
                            # 1/4 rate: widen the last diagonal tile's
                            # streams from 128 to 256 (junk scores in cols
                            # 256:384 are cut by the es memset below)
                            soff = (256 if (diag == 3 and WIDEN
                                            and not ATT_BF16) else toff)
                            scp = psS.tile([128, 1024], F32, tag="sc", bufs=2,
                                           name=f"scp{R}_{n}_{p}_{st}")
                            es = stAB.tile([128, 1024], ADT, tag="es", bufs=3,
                                           name=f"es{R}_{n}_{p}_{st}")
                            for hp in range(2):
                                pb = 64 * hp
                                nc.tensor.matmul(
                                    scp[:, hp * 512 + soff:(hp + 1) * 512],
                                    qk_sb[pb:pb + 64, 2 + p,
                                          st * 128:(st + 1) * 128],
                                    qk_sb[pb:pb + 64, p,
                                          n * 512 + soff:(n + 1) * 512],
                                    start=True, stop=True,
                                )
                            if diag < 0:
                                nc.scalar.activation(
                                    es[:], scp[:], AF.Exp, scale=0.125
                                )
                            else:
                                esv = es[:].rearrange(
                                    "p (hp u) -> p hp u", hp=2)
                                scv = scp[:].rearrange(
                                    "p (hp u) -> p hp u", hp=2)
                                nc.scalar.activation(
                                    esv[:, :, toff:512], scv[:, :, toff:512],
                                    AF.Exp, scale=0.125,
                                )
                                if diag == 3 and WIDEN and not ATT_BF16:
                                    # zero cols 256:384 so the widened
                                    # (fp32r moving>=256 full-rate) att@V
                                    # stream adds nothing to those queries
                                    nc.vector.memset(
                                        esv[:, :, 256:384].bitcast(F32),
                                        0.0)
                                for hp in range(2):
                                    nc.vector.tensor_tensor(
                                        es[:, hp * 512 + toff:
                                           hp * 512 + toff + 128],
                                        es[:, hp * 512 + toff:
                                           hp * 512 + toff + 128],
                                        tri_sb[:], MUL,
                                    )
                            for hp, yp in ((0, ype), (1, ypo)):
                                h = 2 * p + hp
                                nc.tensor.matmul(
                                    yp[0:AVM, soff:512],
                                    v_sb[:, st, h, 0:AVM],
                                    es[:, hp * 512 + soff:(hp + 1) * 512],
                                    start=(st == 0), stop=(st == n_st - 1),
                                )
                        for hp, yp in ((0, ype), (1, ypo)):
                            h = 2 * p + hp
                            if NORM_MODE == "copy":
                                nc.vector.tensor_copy(
                                    ynorm[:, h, :], yp[64:128, :])
                                continue
                            if NORM_MODE == "copy0":
                                nc.vector.tensor_copy(
                                    ynorm[:, h, :], yp[0:64, :])
                                continue
                            if NORM_MODE == "divide":
                                den = stAB.tile([64, 512], F32, tag="rf",
                                                bufs=2, name=f"dn{R}_{n}_{h}")
                                nc.vector.tensor_copy(den[:], yp[0:64, :])
                                nc.vector.tensor_tensor(
                                    ynorm[:, h, :], yp[64:128, :], den[:],
                                    DIV,
                                )
                                continue
                            rf = stAB.tile([64, 512], F32, tag="rf", bufs=2,
                                           name=f"rf{R}_{n}_{h}")
                            if NORM_MODE == "recip":
                                nc.vector.reciprocal(rf[:], yp[0:64, :])
                            else:
                                nc.vector.reciprocal_approx_fast(
                                    rf[:], yp[0:64, :])
                            nc.vector.tensor_tensor(
                                ynorm[:, h, :], yp[64:128, :], rf[:], MUL,
                            )
                    pending = (n, ynorm)

            # free the attention score/accumulator banks (6), keep psA
            # open; proj 0-2 draws only from the freed space, so it is NOT
            # gated on the last chunk's AllGather.
            sy_ctx.close()

            def emit_proj(q, psP):
                pp0 = psP.tile([128, 512], F32, tag="pp0", bufs=2,
                               name=f"pp0{R}_{q}")
                pp1 = psP.tile([128, 512], F32, tag="pp1", bufs=2,
                               name=f"pp1{R}_{q}")
                for kc in range(NKC):
                    yf = stAB.tile([128, 512], ADT, tag="yf", bufs=4,
                                   name=f"yf{R}_{q}_{kc}")
                    if q < 3:
                        dma_eng = nc.sync
                    else:
                        dma_eng = nc.sync if kc % 2 == 0 else nc.scalar
                    dma_eng.dma_start(
                        yf[:], y_full_q[q][kc * 128:(kc + 1) * 128, :]
                    )
                    for m2, pp in ((0, pp0), (1, pp1)):
                        nc.tensor.matmul(
                            pp[:],
                            wp_sb[:, kc, m2 * 128:(m2 + 1) * 128],
                            yf[:],
                            start=(kc == 0), stop=(kc == NKC - 1),
                        )
                out_sb = stAB.tile([128, 2, 512], F32, tag="out_sb", bufs=2,
                                   name=f"out_sb{R}_{q}")
                for m2, pp in ((0, pp0), (1, pp1)):
                    if bout_sb is not None:
                        nc.scalar.activation(
                            out_sb[:, m2, :], pp[:], AF.Copy,
                            bias=bout_sb[:, m2:m2 + 1],
                        )
                    else:
                        nc.vector.tensor_copy(out_sb[:, m2, :], pp[:])
                nc.sync.dma_start(
                    out_c[:, q * 512:(q + 1) * 512].rearrange(
                        "(m p) t -> p m t", p=128),
                    out_sb[:],
                )

            with tc.tile_pool(name=f"psP{R}", bufs=1, space="PSUM") as psP:
                for q in range(3):
                    emit_proj(q, psP)
                emit_ag(*pending)
                emit_proj(3, psP)

def _chunked(a):
    """(C, X) -> [128, C/128, X] contraction-chunked layout."""
    c, x = a.shape
    return np.ascontiguousarray(
        a.reshape(c // 128, 128, x).transpose(1, 0, 2)
    )


def make_in_maps(x, w_attn, b_attn, w_proj, b_proj):
    x = np.asarray(x, dtype=np.float32)
    w_attn = np.asarray(w_attn, dtype=np.float32)
    b_attn = np.asarray(b_attn, dtype=np.float32)
    w_proj = np.asarray(w_proj, dtype=np.float32)
    b_proj = np.asarray(b_proj, dtype=np.float32)

    qk_bias = bool(np.any(b_attn[: 2 * C] != 0))
    b_out_full = b_attn[2 * C:] @ w_proj + b_proj  # V bias folds through
    out_bias = bool(np.any(b_out_full != 0))

    import ml_dtypes
    adt = ml_dtypes.bfloat16 if ATT_BF16 else np.float32
    qdt = ml_dtypes.bfloat16 if QKV_BF16 else np.float32
    tri_np = np.triu(np.ones((128, 128), np.float32)).astype(adt)
    xt_g = []
    for g in range(B):
        xt_g.append(_chunked(np.ascontiguousarray(x[g].T)).astype(qdt))

    in_maps = []
    for core in range(NCORES):
        g, r = core // 4, core % 4
        h0 = r * HL
        qcols = slice(h0 * D, (h0 + HL) * D)
        kcols = slice(C + h0 * D, C + (h0 + HL) * D)
        vcols = slice(2 * C + h0 * D, 2 * C + (h0 + HL) * D)
        wqk_np = _chunked(np.concatenate(
            [w_attn[:, qcols], w_attn[:, kcols]], axis=1)).astype(qdt)
        wv_np = _chunked(np.ascontiguousarray(w_attn[:, vcols])).astype(qdt)
        wp_np = _chunked(np.ascontiguousarray(
            w_proj[:, 256 * r: 256 * (r + 1)])).astype(adt)
        m = {
            "xt": xt_g[g],
            "wqk": wqk_np,
            "wv": wv_np,
            "wp": wp_np,
            "tri": tri_np,
        }
        if qk_bias:
            bq = np.concatenate([b_attn[qcols], b_attn[kcols]])  # (512,)
            m["bqk"] = np.ascontiguousarray(
                bq.reshape(4, 128).T.astype(np.float32))
        if out_bias:
            bo = b_out_full[256 * r: 256 * (r + 1)]
            m["bout"] = np.ascontiguousarray(
                bo.reshape(2, 128).T.astype(np.float32))
        in_maps.append(m)
    return in_maps, qk_bias, out_bias


def assemble_output(results):
    out = np.empty((B, T, C), dtype=np.float32)
    for core in range(NCORES):
        g, r = core // 4, core % 4
        out[g][:, 256 * r: 256 * (r + 1)] = results[core]["out_c"].T
    return out


def kernel(x, w_attn, b_attn, w_proj, b_proj):
    from concourse.bass_utils import run_bass_kernel_spmd

    in_maps, qk_bias, out_bias = make_in_maps(
        x, w_attn, b_attn, w_proj, b_proj)
    nc = build_program(reps=1, qk_bias=qk_bias, out_bias=out_bias)
    res = run_bass_kernel_spmd(nc, in_maps, list(range(NCORES)))
    return assemble_output(res.results)


# revision 31
# speedup vs baseline: 1.4916x; 1.4916x over previous
"""Causal self-attention (B=2, T=2048, C=1024, H=16, D=64) on 8 TRN2 cores.

Sharding: batch across 2 groups of 4 cores; 4 heads per core within a group
(Megatron column-parallel QKV). After attention, AllGather the per-head
outputs within each group, then column-parallel c_proj (each core computes
256 output columns for all T), so the device program is rank-independent.

Per-core dataflow (bf16 operands throughout — measured ~35% faster than
fp32r on hw A/B despite the explicit Ldweights the toolchain emits for
non-fp32 matmuls; fp32r variants remain selectable via module flags):
  xt  [128, 8, 2048]  x[b]^T chunked by contraction (C) blocks
  Q^T/K^T computed as [512 rows, T] (lhsT = w_qk slices, rhs = xt).
  V computed natural [T, 4 heads x 128] where columns 0:64 of each head's
  block are ONES (memset) — the att@V matmul then emits the softmax
  denominator replicated across psum rows 0:64 and y^T in rows 64:128, so
  normalization is a pure-DVE reciprocal + fused multiply with no PE
  broadcast matmuls and no denominator DMA round-trip (the ones come
  first so the denominator sits at psum base partition 0, where the
  base-partition-matched DVE reciprocal can read it; reading psum across
  base partitions is legal for plain DVE ops but silently wrong for the
  custom-DVE reciprocal).
  S^T block matmuls (K=64) row-paired across head pairs (partitions 0-63 /
  64-127), exp on ACT (scale=1/8 fused) written to bf16, triangular mask
  on diagonal 128-blocks. In fp32r mode, matmuls with moving dim < 256
  run at 1/4 rate, so the last diagonal tile's score/att@V streams are
  widened from 128 to 256 (junk score columns zeroed in es first).

QKV (stage A) and attention (stage B) are emitted interleaved per t-chunk
so the Tile scheduler can fill PE gaps during ACT exp with next-chunk QKV
matmuls. Each t-chunk's DMA+AllGather is deferred by one chunk and issued
mid-compute so the first three AllGathers hide under attention. The
score/accumulator PSUM pools close right after attention so proj quarters
0-2 run before the final AllGather, leaving only quarter 3's proj on the
exposed tail.

Output per core: out_c [256, 2048] = out^T columns slice; host reassembles.
"""

import sys

sys.path.insert(0, "/opt/trn_rl_repo")

from contextlib import ExitStack

import numpy as np

B, T, C, H, D = 2, 2048, 1024, 16, 64
NCORES = 8
HL = 4  # heads per core
NKC = 8  # contraction chunks (C / 128)
NCH = 4  # t chunks (T / 512)
NST = 16  # s tiles (T / 128)

_prog_cache = {}

# When True, AllGather is replaced by local DRAM copies (same deps/bytes
# shape) so the single-core TimelineSim can analyze the program.
SIM_NO_COLLECTIVE = False

# Normalize implementation: 'approx' (custom-DVE reciprocal + mult),
# 'recip' (native DVE reciprocal + mult), 'divide' (den copy + ALU divide),
# 'copy' (no normalize — WRONG RESULTS, timing probe only).
NORM_MODE = "approx"

# Widen diag-3 fp32r streams from 128 to 256 (with es zero-memset).
WIDEN = 1

# Timing probes: att@V stationary columns (128 = ones+V, 65 = probe);
# skip the v_sb ones memset (wrong results, timing probe).
AVM = 128
MEMSET = 1

# bf16 attention: qk/es/v/y/proj-weight tensors in bf16 (QKV and V
# projections stay fp32r). Measured faster than fp32r on hw despite
# the explicit Ldweights the toolchain emits for non-fp32 matmuls.
ATT_BF16 = 1

# bf16 QKV/V projections too: xt, wqk, wv in bf16 (halves their SBUF
# streams and DMA). Costs ~0.5% relative error on q/k/v.
QKV_BF16 = 1



def build_program(reps=1, qk_bias=False, out_bias=False):
    key = (reps, qk_bias, out_bias, SIM_NO_COLLECTIVE, NORM_MODE, WIDEN,
           AVM, MEMSET, ATT_BF16, QKV_BF16)
    if key in _prog_cache:
        return _prog_cache[key]

    from concourse import bacc, mybir
    import concourse.tile as tile

    F32 = mybir.dt.float32
    F32R = mybir.dt.float32r
    BF16 = mybir.dt.bfloat16

    nc = bacc.Bacc(num_devices=NCORES)

    ADT = BF16 if ATT_BF16 else F32R
    QDT = BF16 if QKV_BF16 else F32R
    xt = nc.declare_dram_parameter("xt", [128, NKC, T], QDT, isOutput=False)
    wqk = nc.declare_dram_parameter("wqk", [128, NKC, 512], QDT, isOutput=False)
    wv = nc.declare_dram_parameter("wv", [128, NKC, 256], QDT, isOutput=False)
    wp = nc.declare_dram_parameter("wp", [128, NKC, 256], ADT, isOutput=False)
    tri = nc.declare_dram_parameter("tri", [128, 128], ADT, isOutput=False)
    if qk_bias:
        bqk = nc.declare_dram_parameter("bqk", [128, 4], F32, isOutput=False)
    if out_bias:
        bout = nc.declare_dram_parameter("bout", [128, 2], F32, isOutput=False)
    out_c = nc.declare_dram_parameter("out_c", [256, T], F32, isOutput=True)

    with tile.TileContext(nc) as tc:
        with ExitStack() as outer:
            const = outer.enter_context(tc.tile_pool(name="const", bufs=1))
            wqk_sb = const.tile([128, NKC, 512], QDT)
            wv_sb = const.tile([128, NKC, 256], QDT)
            wp_sb = const.tile([128, NKC, 256], ADT)
            tri_sb = const.tile([128, 128], ADT)
            nc.scalar.dma_start(wqk_sb[:], wqk[:])
            nc.scalar.dma_start(wv_sb[:], wv[:])
            nc.scalar.dma_start(wp_sb[:], wp[:])
            nc.scalar.dma_start(tri_sb[:], tri[:])
            bqk_sb = bout_sb = None
            if qk_bias:
                bqk_sb = const.tile([128, 4], F32)
                nc.scalar.dma_start(bqk_sb[:], bqk[:])
            if out_bias:
                bout_sb = const.tile([128, 2], F32)
                nc.scalar.dma_start(bout_sb[:], bout[:])

            for rep in range(reps):
                _emit_body(
                    nc, tc, mybir, rep,
                    xt=xt, out_c=out_c,
                    wqk_sb=wqk_sb, wv_sb=wv_sb, wp_sb=wp_sb, tri_sb=tri_sb,
                    bqk_sb=bqk_sb, bout_sb=bout_sb,
                )

    nc.finalize()
    _prog_cache[key] = nc
    return nc


def _emit_body(nc, tc, mybir, rep, *, xt, out_c, wqk_sb, wv_sb, wp_sb,
               tri_sb, bqk_sb, bout_sb):
    F32 = mybir.dt.float32
    F32R = mybir.dt.float32r
    BF16 = mybir.dt.bfloat16
    ADT = BF16 if ATT_BF16 else F32R
    QDT = BF16 if QKV_BF16 else F32R
    AF = mybir.ActivationFunctionType
    MUL = mybir.AluOpType.mult
    R = f"r{rep}"

    with ExitStack() as persist:
        stP = persist.enter_context(tc.tile_pool(name=f"stP{R}", bufs=1))
        dpool = persist.enter_context(
            tc.tile_pool(name=f"dram{R}", bufs=1, space="DRAM"))
        # Q^T/K^T: m-tiles 0,1 = Q pairs; 2,3 = K pairs. [128, m, t]
        qk_sb = stP.tile([128, 4, T], ADT, name=f"qk_sb{R}")
        # V natural, per-rep (a shared tile would serialize reps); per head
        # 128 cols: 0:64 = ones (softmax denominator via the att@V matmul,
        # landing at psum base partition 0 where the base-matched DVE
        # reciprocal can read it), 64:128 = V values.
        v_sb = stP.tile([128, NST, 4, 128], ADT, name=f"v_sb{R}")
        y_in_q = [
            dpool.tile([256, 512], ADT, name=f"y_in{R}_{q}")
            for q in range(NCH)
        ]
        y_full_q = [
            dpool.tile([1024, 512], ADT, name=f"y_full{R}_{q}")
            for q in range(NCH)
        ]

        with (
            tc.tile_pool(name=f"stAB{R}", bufs=1) as stAB,
            tc.tile_pool(name=f"psA{R}", bufs=1, space="PSUM") as psA,
        ):
            sy_ctx = ExitStack()
            psS = sy_ctx.enter_context(
                tc.tile_pool(name=f"psS{R}", bufs=1, space="PSUM"))
            psY = sy_ctx.enter_context(
                tc.tile_pool(name=f"psY{R}", bufs=1, space="PSUM"))
            if MEMSET:
                ones = v_sb[:, :, :, 0:64]
                nc.vector.memset(
                    ones if ATT_BF16 else ones.bitcast(F32), 1.0)
            xt_t = []
            for n in range(NCH):
                xtile = stAB.tile([128, NKC, 512], QDT, tag="xt", bufs=4,
                                  name=f"xt_t{R}_{n}")
                nc.sync.dma_start(
                    xtile[:], xt[:, :, n * 512:(n + 1) * 512])
                xt_t.append(xtile)

            def emit_ag(n, ynorm):
                nc.scalar.dma_start(
                    y_in_q[n][:].rearrange("(h p) u -> p h u", p=64),
                    ynorm[:],
                )
                if SIM_NO_COLLECTIVE:
                    for r in range(4):
                        nc.gpsimd.dma_start(
                            y_full_q[n][r * 256:(r + 1) * 256, :],
                            y_in_q[n][:],
                        )
                else:
                    nc.gpsimd.collective_compute(
                        "AllGather",
                        mybir.AluOpType.bypass,
                        replica_groups=[[0, 1, 2, 3], [4, 5, 6, 7]],
                        ins=[y_in_q[n][:]],
                        outs=[y_full_q[n][:]],
                    )

            pending = None
            for n in range(NCH):
                    # normalized y^T, heads on free dim: [64 d, 4 h, 512 t];
                    # written this chunk, DMA'd next chunk (2 rotating bufs)
                    ynorm = stAB.tile([64, 4, 512], ADT, tag="yn", bufs=2,
                                      name=f"ynorm{R}_{n}")
                    for m in range(4):
                        ps = psA.tile([128, 512], F32, tag="pA", bufs=2,
                                      name=f"qkvps{R}_{n}_{m}")
                        for kc in range(NKC):
                            nc.tensor.matmul(
                                ps[:],
                                wqk_sb[:, kc, m * 128:(m + 1) * 128],
                                xt_t[n][:, kc, :],
                                start=(kc == 0), stop=(kc == NKC - 1),
                            )
                        if bqk_sb is not None:
                            nc.scalar.activation(
                                qk_sb[:, m, n * 512:(n + 1) * 512], ps[:],
                                AF.Copy, bias=bqk_sb[:, m:m + 1],
                            )
                        else:
                            nc.vector.tensor_copy(
                                qk_sb[:, m, n * 512:(n + 1) * 512], ps[:]
                            )
                    for tt in range(4 * n, 4 * n + 4):
                        psv = psA.tile([128, 512], F32, tag="pA", bufs=2,
                                       name=f"vps{R}_{tt}")
                        for kc in range(NKC):
                            nc.tensor.matmul(
                                psv[:, 0:256],
                                xt_t[n][:, kc,
                                        (tt - 4 * n) * 128:
                                        (tt - 4 * n + 1) * 128],
                                wv_sb[:, kc, :],
                                start=(kc == 0), stop=(kc == NKC - 1),
                            )
                        nc.vector.tensor_copy(
                            v_sb[:, tt, :, 64:128],
                            psv[:, 0:256].rearrange("p (h x) -> p h x", h=4),
                        )

                    if pending is not None:
                        emit_ag(*pending)
                        pending = None

                    n_st = 4 * n + 4
                    for p in range(2):
                        ype = psY.tile([128, 512], F32, tag="ye", bufs=1,
                                       name=f"ype{R}_{n}_{p}")
                        ypo = psY.tile([128, 512], F32, tag="yo", bufs=1,
                                       name=f"ypo{R}_{n}_{p}")
                        for st in range(n_st):
                            diag = st - 4 * n
                            toff = 128 * diag if diag >= 0 else 0
                            # fp32r matmuls with moving dim < 256 run at
                            # 1/4 rate: widen the last diagonal tile's
                            # streams from 128 to 256 (junk scores in cols
                            # 256:384 are cut by the es memset below)
                            soff = (256 if (diag == 3 and WIDEN
                                            and not ATT_BF16) else toff)
                            scp = psS.tile([128, 1024], F32, tag="sc", bufs=2,
                                           name=f"scp{R}_{n}_{p}_{st}")
                            es = stAB.tile([128, 1024], ADT, tag="es", bufs=4,
                                           name=f"es{R}_{n}_{p}_{st}")
                            for hp in range(2):
                                pb = 64 * hp
                                nc.tensor.matmul(
                                    scp[:, hp * 512 + soff:(hp + 1) * 512],
                                    qk_sb[pb:pb + 64, 2 + p,
                                          st * 128:(st + 1) * 128],
                                    qk_sb[pb:pb + 64, p,
                                          n * 512 + soff:(n + 1) * 512],
                                    start=True, stop=True,
                                )
                            if diag < 0:
                                nc.scalar.activation(
                                    es[:], scp[:], AF.Exp, scale=0.125
                                )
                            else:
                                esv = es[:].rearrange(
                                    "p (hp u) -> p hp u", hp=2)
                                scv = scp[:].rearrange(
                                    "p (hp u) -> p hp u", hp=2)
                                nc.scalar.activation(
                                    esv[:, :, toff:512], scv[:, :, toff:512],
                                    AF.Exp, scale=0.125,
                                )
                                if diag == 3 and WIDEN and not ATT_BF16:
                                    # zero cols 256:384 so the widened
                                    # (fp32r moving>=256 full-rate) att@V
                                    # stream adds nothing to those queries
                                    nc.vector.memset(
                                        esv[:, :, 256:384].bitcast(F32),
                                        0.0)
                                for hp in range(2):
                                    nc.vector.tensor_tensor(
                                        es[:, hp * 512 + toff:
                                           hp * 512 + toff + 128],
                                        es[:, hp * 512 + toff:
                                           hp * 512 + toff + 128],
                                        tri_sb[:], MUL,
                                    )
                            for hp, yp in ((0, ype), (1, ypo)):
                                h = 2 * p + hp
                                nc.tensor.matmul(
                                    yp[0:AVM, soff:512],
                                    v_sb[:, st, h, 0:AVM],
                                    es[:, hp * 512 + soff:(hp + 1) * 512],
                                    start=(st == 0), stop=(st == n_st - 1),
                                )
                        for hp, yp in ((0, ype), (1, ypo)):
                            h = 2 * p + hp
                            if NORM_MODE == "copy":
                                nc.vector.tensor_copy(
                                    ynorm[:, h, :], yp[64:128, :])
                                continue
                            if NORM_MODE == "copy0":
                                nc.vector.tensor_copy(
                                    ynorm[:, h, :], yp[0:64, :])
                                continue
                            if NORM_MODE == "divide":
                                den = stAB.tile([64, 512], F32, tag="rf",
                                                bufs=2, name=f"dn{R}_{n}_{h}")
                                nc.vector.tensor_copy(den[:], yp[0:64, :])
                                nc.vector.tensor_tensor(
                                    ynorm[:, h, :], yp[64:128, :], den[:],
                                    DIV,
                                )
                                continue
                            rf = stAB.tile([64, 512], F32, tag="rf", bufs=2,
                                           name=f"rf{R}_{n}_{h}")
                            if NORM_MODE == "recip":
                                nc.vector.reciprocal(rf[:], yp[0:64, :])
                            else:
                                nc.vector.reciprocal_approx_fast(
                                    rf[:], yp[0:64, :])
                            nc.vector.tensor_tensor(
                                ynorm[:, h, :], yp[64:128, :], rf[:], MUL,
                            )
                    pending = (n, ynorm)

            # free the attention score/accumulator banks (6), keep psA
            # open; proj 0-2 draws only from the freed space, so it is NOT
            # gated on the last chunk's AllGather.
            sy_ctx.close()

            def emit_proj(q, psP):
                pp0 = psP.tile([128, 512], F32, tag="pp0", bufs=2,
                               name=f"pp0{R}_{q}")
                pp1 = psP.tile([128, 512], F32, tag="pp1", bufs=2,
                               name=f"pp1{R}_{q}")
                for kc in range(NKC):
                    yf = stAB.tile([128, 512], ADT, tag="yf", bufs=6,
                                   name=f"yf{R}_{q}_{kc}")
                    if q < 3:
                        dma_eng = nc.sync
                    else:
                        dma_eng = nc.sync if kc % 2 == 0 else nc.scalar
                    dma_eng.dma_start(
                        yf[:], y_full_q[q][kc * 128:(kc + 1) * 128, :]
                    )
                    for m2, pp in ((0, pp0), (1, pp1)):
                        nc.tensor.matmul(
                            pp[:],
                            wp_sb[:, kc, m2 * 128:(m2 + 1) * 128],
                            yf[:],
                            start=(kc == 0), stop=(kc == NKC - 1),
                        )
                out_sb = stAB.tile([128, 2, 512], F32, tag="out_sb", bufs=2,
                                   name=f"out_sb{R}_{q}")
                for m2, pp in ((0, pp0), (1, pp1)):
                    if bout_sb is not None:
                        nc.scalar.activation(
                            out_sb[:, m2, :], pp[:], AF.Copy,
                            bias=bout_sb[:, m2:m2 + 1],
                        )
                    else:
                        nc.vector.tensor_copy(out_sb[:, m2, :], pp[:])
                nc.sync.dma_start(
                    out_c[:, q * 512:(q + 1) * 512].rearrange(
                        "(m p) t -> p m t", p=128),
                    out_sb[:],
                )

            with tc.tile_pool(name=f"psP{R}", bufs=1, space="PSUM") as psP:
                for q in range(3):
                    emit_proj(q, psP)
                emit_ag(*pending)
                emit_proj(3, psP)

def _chunked(a):
    """(C, X) -> [128, C/128, X] contraction-chunked layout."""
    c, x = a.shape
    return np.ascontiguousarray(
        a.reshape(c // 128, 128, x).transpose(1, 0, 2)
    )


def make_in_maps(x, w_attn, b_attn, w_proj, b_proj):
    x = np.asarray(x, dtype=np.float32)
    w_attn = np.asarray(w_attn, dtype=np.float32)
    b_attn = np.asarray(b_attn, dtype=np.float32)
    w_proj = np.asarray(w_proj, dtype=np.float32)
    b_proj = np.asarray(b_proj, dtype=np.float32)

    qk_bias = bool(np.any(b_attn[: 2 * C] != 0))
    b_out_full = b_attn[2 * C:] @ w_proj + b_proj  # V bias folds through
    out_bias = bool(np.any(b_out_full != 0))

    import ml_dtypes
    adt = ml_dtypes.bfloat16 if ATT_BF16 else np.float32
    qdt = ml_dtypes.bfloat16 if QKV_BF16 else np.float32
    tri_np = np.triu(np.ones((128, 128), np.float32)).astype(adt)
    xt_g = []
    for g in range(B):
        xt_g.append(_chunked(np.ascontiguousarray(x[g].T)).astype(qdt))

    in_maps = []
    for core in range(NCORES):
        g, r = core // 4, core % 4
        h0 = r * HL
        qcols = slice(h0 * D, (h0 + HL) * D)
        kcols = slice(C + h0 * D, C + (h0 + HL) * D)
        vcols = slice(2 * C + h0 * D, 2 * C + (h0 + HL) * D)
        wqk_np = _chunked(np.concatenate(
            [w_attn[:, qcols], w_attn[:, kcols]], axis=1)).astype(qdt)
        wv_np = _chunked(np.ascontiguousarray(w_attn[:, vcols])).astype(qdt)
        wp_np = _chunked(np.ascontiguousarray(
            w_proj[:, 256 * r: 256 * (r + 1)])).astype(adt)
        m = {
            "xt": xt_g[g],
            "wqk": wqk_np,
            "wv": wv_np,
            "wp": wp_np,
            "tri": tri_np,
        }
        if qk_bias:
            bq = np.concatenate([b_attn[qcols], b_attn[kcols]])  # (512,)
            m["bqk"] = np.ascontiguousarray(
                bq.reshape(4, 128).T.astype(np.float32))
        if out_bias:
            bo = b_out_full[256 * r: 256 * (r + 1)]
            m["bout"] = np.ascontiguousarray(
                bo.reshape(2, 128).T.astype(np.float32))
        in_maps.append(m)
    return in_maps, qk_bias, out_bias


def assemble_output(results):
    out = np.empty((B, T, C), dtype=np.float32)
    for core in range(NCORES):
        g, r = core // 4, core % 4
        out[g][:, 256 * r: 256 * (r + 1)] = results[core]["out_c"].T
    return out


def kernel(x, w_attn, b_attn, w_proj, b_proj):
    from concourse.bass_utils import run_bass_kernel_spmd

    in_maps, qk_bias, out_bias = make_in_maps(
        x, w_attn, b_attn, w_proj, b_proj)
    nc = build_program(reps=1, qk_bias=qk_bias, out_bias=out_bias)
    res = run_bass_kernel_spmd(nc, in_maps, list(range(NCORES)))
    return assemble_output(res.results)
